# revision 8
# baseline (speedup 1.0000x reference)
"""Causal self-attention (B=4, T=2048, D=1024, H=16) on 8 TRN2 NeuronCores.

Sharding: core (b, g) with b in 0..3, g in 0..1 handles batch b, heads
[g*8, g*8+8). Each core computes its heads' qkv projection, causal
attention, and a partial out-projection (columns g*512..g*512+512 of the
contraction); the host sums the two partials per batch.

All matmuls run in float32r (single-pass fp32, ~tf32 precision, 1 cyc/row).
Layouts avoid every transpose: host supplies x^T and pre-transposed weight
slices; S is computed transposed ([k, q]) so softmax normalization becomes a
column sum, obtained for free via a ones-column in the AV matmul (M=65).
"""

import numpy as np

import bass_rust
import concourse.bass as bass
import concourse.mybir as mybir
import concourse.tile as tile
from concourse.bass_utils import run_bass_kernel_spmd

f32 = mybir.dt.float32
f32r = mybir.dt.float32r
Exp = mybir.ActivationFunctionType.Exp

B, T, D = 4, 2048, 1024
H, DH = 16, 64
HC = 8          # heads per core
OC = 3 * 512    # qkv rows per core
P = 128
NQC = T // 512  # q chunks
NKB = T // 128  # k blocks


# ---------------------------------------------------------------------------
# Compat patches: this container's walrus accepts only ONE sync wait per
# instruction, while Tile freely attaches several. Split excess waits onto
# injected same-engine NOPs / extra drains.
# ---------------------------------------------------------------------------

def _drain_and_barrier_split(self, tick_clock, wait_clock):
    nc = self.nc
    drain_inst = nc.sync.drain()
    wait_clock.add_sem_waits(
        drain_inst.ins, bass_rust.ScopedClock({None: tick_clock.global_clock})
    )
    si = drain_inst.ins.sync_info
    waits = list(si.on_wait or []) if si is not None else []
    if len(waits) > 1:
        si.on_wait = waits[:1]
        for w in waits[1:]:
            extra = nc.sync.drain()
            esi = extra.ins.sync_info
            if esi is None:
                extra.ins.sync_info = bass_rust.SyncInfo(on_wait=[w], on_update=[])
            else:
                esi.on_wait = [w]

    nc.all_engine_barrier()
    assert self.sems is not None
    popped = nc._tile_sem_poison_stack.pop()
    assert popped is self._sem_poison
    nc.clear_and_free_semaphores(list(self.sems.allocated().values()))
    nc.all_engine_barrier()


def _apply_compat_patches():
    tile.TileContext._drain_and_barrier = _drain_and_barrier_split


def _split_excess_waits(nc):
    for fn in nc.m.functions:
        for bb in fn.blocks:
            out = []
            changed = False
            for inst in bb.instructions:
                si = inst.sync_info
                if si is not None and si.on_wait and len(si.on_wait) > 1:
                    waits = list(si.on_wait)
                    for w in waits[:-1]:
                        nop = mybir.InstNoOp(
                            name=f"I-waitsplit-{nc.next_id()}",
                            engine=inst.engine,
                            ins=[],
                            outs=[],
                            sync_info=bass_rust.SyncInfo(on_wait=[w], on_update=[]),
                        )
                        nc.register_instruction(nop)
                        out.append(nop)
                    si.on_wait = waits[-1:]
                    changed = True
                out.append(inst)
            if changed:
                bb.instructions = out


# ---------------------------------------------------------------------------
# Kernel program (identical for all 8 cores; only the bound data differs)
# ---------------------------------------------------------------------------

def _build_nc():
    nc = bass.Bass()
    xT = nc.dram_tensor("xT", [D, T], f32, kind="ExternalInput")
    wqkvT = nc.dram_tensor("wqkvT", [D, OC], f32, kind="ExternalInput")
    woT = nc.dram_tensor("woT", [512, D], f32, kind="ExternalInput")
    trimask = nc.dram_tensor("trimask", [P, P], f32, kind="ExternalInput")
    outT = nc.dram_tensor("outT", [D, T], f32, kind="ExternalOutput")

    xT_r = xT.rearrange("(cs p) t -> p cs t", p=P)          # [128, 8, 2048]
    wqkvT_r = wqkvT.rearrange("(cs p) o -> p cs o", p=P)    # [128, 8, 1536]
    woT_r = woT.rearrange("(cs p) o -> p cs o", p=P)        # [128, 4, 1024]
    outT_r = outT.rearrange("(ob p) t -> p ob t", p=P)      # [128, 8, 2048]

    XC = 256            # t-chunk width for the qkv projection
    NXC = T // XC

    with tile.TileContext(nc) as tc:
        # pool_a: tensors produced in phase 1, consumed in phase 2
        pool_a_cm = tc.tile_pool(name="pool_a", bufs=1)
        pool_a = pool_a_cm.__enter__()
        qT = pool_a.tile([P, HC // 2, T], f32r)      # [128, hp, t] head pair hp
        kT = pool_a.tile([P, HC // 2, T], f32r)
        V = pool_a.tile([P, NKB, HC, DH + 1], f32r)  # [k-part, kb, h, d|1]
        maskr = pool_a.tile([P, P], f32r)

        # ---- phase 0: constants -------------------------------------------
        with tc.tile_pool(name="ph0", bufs=1) as ph0:
            mf = ph0.tile([P, P], f32)
            nc.sync.dma_start(mf[:, :], trimask[:, :])
            nc.vector.tensor_copy(maskr[:, :], mf[:, :])
            onef = ph0.tile([P, 1], f32)
            nc.vector.memset(onef[:, :], 1.0)
            # ones column of V: broadcast along kb/h
            nc.vector.tensor_copy(
                V[:, :, :, DH : DH + 1],
                onef[:, None, None, :].to_broadcast((P, NKB, HC, 1)),
            )

        # ---- phase 1: qkv projections -------------------------------------
        with (
            tc.tile_pool(name="wstage", bufs=1) as wstage,
            tc.tile_pool(name="wpool", bufs=1) as wpool,
            tc.tile_pool(name="xpool", bufs=2) as xpool,
            tc.tile_pool(name="ppsum", bufs=3, space="PSUM") as ppsum,
        ):
            wr = wpool.tile([P, D // P, OC], f32r)
            for i in range(4):
                ws = wstage.tile([P, D // P, OC // 4], f32, tag="ws")
                nc.sync.dma_start(ws[:, :, :], wqkvT_r[:, :, i * (OC // 4):(i + 1) * (OC // 4)])
                nc.vector.tensor_copy(wr[:, :, i * (OC // 4):(i + 1) * (OC // 4)], ws[:, :, :])

            for tcb in range(NXC):
                tlo = tcb * XC
                xs = xpool.tile([P, D // P, XC], f32, tag="xs")
                nc.sync.dma_start(xs[:, :, :], xT_r[:, :, tlo:tlo + XC])
                xr = xpool.tile([P, D // P, XC], f32r, tag="xr")
                nc.vector.tensor_copy(xr[:, :, :], xs[:, :, :])

                # q, k in [o, t] orientation: out rows = features
                for ob in range(8):
                    ps = ppsum.tile([P, XC], f32, tag="pp")
                    for ci in range(D // P):
                        nc.tensor.matmul(
                            ps[:, :],
                            lhsT=wr[:, ci, ob * P:(ob + 1) * P],
                            rhs=xr[:, ci, :],
                            start=(ci == 0),
                            stop=(ci == D // P - 1),
                        )
                    dst = qT if ob < 4 else kT
                    nc.scalar.copy(dst[:, ob % 4, tlo:tlo + XC], ps[:, :])

                # v in [t, o] orientation: out rows = tokens
                for ts in range(XC // P):
                    kb = (tlo + ts * P) // P
                    ps = ppsum.tile([P, 512], f32, tag="pv")
                    for ci in range(D // P):
                        nc.tensor.matmul(
                            ps[:, :],
                            lhsT=xr[:, ci, ts * P:(ts + 1) * P],
                            rhs=wr[:, ci, 1024:1536],
                            start=(ci == 0),
                            stop=(ci == D // P - 1),
                        )
                    nc.vector.tensor_copy(
                        V[:, kb, :, 0:DH],
                        ps.rearrange("p (h d) -> p h d", h=HC),
                    )

        # ---- phase 2: causal attention ------------------------------------
        # persist: tensors needed through phase 3 (opened after phase-1 pools
        # close so yT reuses their SBUF space)
        persist_cm = tc.tile_pool(name="persist", bufs=1)
        persist = persist_cm.__enter__()
        yT = persist.tile([P, HC // 2, T], f32r)
        ones1 = persist.tile([1, DH], f32r)

        with (
            tc.tile_pool(name="ptpool", bufs=3) as ptpool,
            tc.tile_pool(name="bcpool", bufs=2) as bcpool,
            tc.tile_pool(name="rzpool", bufs=2) as rzpool,
            tc.tile_pool(name="spsum", bufs=2, space="PSUM") as spsum,
            tc.tile_pool(name="ypsum", bufs=2, space="PSUM") as ypsum,
            tc.tile_pool(name="bcpsum", bufs=2, space="PSUM") as bcpsum,
        ):
            onef2 = rzpool.tile([1, 1], f32, tag="onef")
            nc.vector.memset(onef2[:, :], 1.0)
            nc.vector.tensor_copy(ones1[:, :], onef2[0:1, 0:1].to_broadcast((1, DH)))
            for hp in range(HC // 2):
                for qc in range(NQC):
                    qlo = qc * 512
                    nkb = 4 * qc + 4
                    psy = [
                        ypsum.tile([DH + 1, 512], f32, tag="psy", name=f"psy_{hp}_{qc}_{hi}")
                        for hi in range(2)
                    ]
                    for kb in range(nkb):
                        j = kb - 4 * qc
                        off = max(0, j * P)
                        ps = spsum.tile([P, 2, 512], f32, tag="ps")
                        for hi in range(2):
                            nc.tensor.matmul(
                                ps[:, hi, off:],
                                lhsT=kT[hi * DH:(hi + 1) * DH, hp, kb * P:(kb + 1) * P],
                                rhs=qT[hi * DH:(hi + 1) * DH, hp, qlo + off:qlo + 512],
                                start=True,
                                stop=True,
                                tile_position=(hi * DH, 0),
                            )
                        pt = ptpool.tile([P, 2, 512], f32r, tag="pt")
                        nc.scalar.activation(pt[:, :, off:], ps[:, :, off:], Exp, scale=0.125)
                        if j >= 0:
                            nc.vector.tensor_mul(
                                pt[:, :, off:off + P],
                                pt[:, :, off:off + P],
                                maskr[:, None, :].to_broadcast((P, 2, P)),
                            )
                        for hi in range(2):
                            h = 2 * hp + hi
                            nc.tensor.matmul(
                                psy[hi][:, off:],
                                lhsT=V[:, kb, h, :],
                                rhs=pt[:, hi, off:],
                                start=(kb == 0),
                                stop=(kb == nkb - 1),
                                skip_group_check=True,
                            )
                    for hi in range(2):
                        rz = rzpool.tile([1, 512], f32r, tag="rz")
                        with nc.allow_low_precision(reason="1/Z rounded for PE broadcast"):
                            nc.vector.reciprocal(rz[:, :], psy[hi][DH:DH + 1, :])
                        bc = bcpsum.tile([DH, 512], f32, tag="bc")
                        nc.tensor.matmul(bc[:, :], lhsT=ones1[:, :], rhs=rz[:, :], start=True, stop=True)
                        bcs = bcpool.tile([DH, 512], f32, tag="bcs")
                        nc.scalar.copy(bcs[:, :], bc[:, :])
                        nc.vector.tensor_mul(
                            yT[hi * DH:(hi + 1) * DH, hp, qlo:qlo + 512],
                            psy[hi][0:DH, :],
                            bcs[:, :],
                        )

        # ---- phase 3: out projection --------------------------------------
        with (
            tc.tile_pool(name="w3stage", bufs=1) as w3stage,
            tc.tile_pool(name="w3pool", bufs=1) as w3pool,
            tc.tile_pool(name="ostage", bufs=2) as ostage,
            tc.tile_pool(name="opsum", bufs=3, space="PSUM") as opsum,
        ):
            w3s = w3stage.tile([P, 4, D], f32)
            nc.sync.dma_start(w3s[:, :, :], woT_r[:, :, :])
            wor = w3pool.tile([P, 4, D], f32r)
            nc.vector.tensor_copy(wor[:, :, :], w3s[:, :, :])

            for tcb in range(NQC):
                tlo = tcb * 512
                ost = ostage.tile([P, 8, 512], f32, tag="ost")
                for ob in range(8):
                    ps = opsum.tile([P, 512], f32, tag="po")
                    for ci in range(4):
                        nc.tensor.matmul(
                            ps[:, :],
                            lhsT=wor[:, ci, ob * P:(ob + 1) * P],
                            rhs=yT[:, ci, tlo:tlo + 512],
                            start=(ci == 0),
                            stop=(ci == 3),
                        )
                    nc.scalar.copy(ost[:, ob, :], ps[:, :])
                nc.sync.dma_start(outT_r[:, :, tlo:tlo + 512], ost[:, :, :])

        persist_cm.__exit__(None, None, None)
        pool_a_cm.__exit__(None, None, None)

    _split_excess_waits(nc)
    return nc


_NC_CACHE = None


def _get_nc():
    global _NC_CACHE
    if _NC_CACHE is None:
        _apply_compat_patches()
        _NC_CACHE = _build_nc()
    return _NC_CACHE


def _shard_inputs(x, w_qkv, w_out):
    trimask = np.triu(np.ones((P, P), dtype=np.float32))
    in_maps = []
    for b in range(B):
        xTb = np.ascontiguousarray(x[b].T)
        for g in range(2):
            wq = w_qkv[g * 512:(g + 1) * 512]
            wk = w_qkv[D + g * 512:D + (g + 1) * 512]
            wv = w_qkv[2 * D + g * 512:2 * D + (g + 1) * 512]
            wqkvT = np.ascontiguousarray(np.concatenate([wq, wk, wv], axis=0).T)
            woT = np.ascontiguousarray(w_out[:, g * 512:(g + 1) * 512].T)
            in_maps.append({"xT": xTb, "wqkvT": wqkvT, "woT": woT, "trimask": trimask})
    return in_maps


def kernel(x, w_qkv, w_out, _trace=False):
    x = np.asarray(x, dtype=np.float32)
    w_qkv = np.asarray(w_qkv, dtype=np.float32)
    w_out = np.asarray(w_out, dtype=np.float32)

    nc = _get_nc()
    in_maps = _shard_inputs(x, w_qkv, w_out)
    res = run_bass_kernel_spmd(nc, in_maps, core_ids=list(range(8)), trace=_trace)

    out = np.empty((B, T, D), dtype=np.float32)
    for b in range(B):
        acc = res.results[2 * b]["outT"] + res.results[2 * b + 1]["outT"]
        out[b] = acc.T
    if _trace:
        return out, res
    return out


# revision 9
# speedup vs baseline: 1.0525x; 1.0525x over previous
"""Causal self-attention (B=4, T=2048, D=1024, H=16) on 8 TRN2 NeuronCores.

Sharding: core (b, g) with b in 0..3, g in 0..1 handles batch b, heads
[g*8, g*8+8). Each core computes its heads' qkv projection, causal
attention, and a partial out-projection (columns g*512..g*512+512 of the
contraction); the host sums the two partials per batch.

All matmuls run in float32r (single-pass fp32, ~tf32 precision, 1 cyc/row).
Layouts avoid every transpose: host supplies x^T and pre-transposed weight
slices; S is computed transposed ([k, q]) so softmax normalization becomes a
column sum, obtained for free via a ones-column in the AV matmul (M=65).
"""

import numpy as np

import bass_rust
import concourse.bass as bass
import concourse.mybir as mybir
import concourse.tile as tile
from concourse.bass_utils import run_bass_kernel_spmd

f32 = mybir.dt.float32
f32r = mybir.dt.float32r
bf16 = mybir.dt.bfloat16
import os as _os
MDT = bf16 if _os.environ.get("K_DTYPE", "f32r") == "bf16" else f32r
Exp = mybir.ActivationFunctionType.Exp

B, T, D = 4, 2048, 1024
H, DH = 16, 64
HC = 8          # heads per core
OC = 3 * 512    # qkv rows per core
P = 128
NQC = T // 512  # q chunks
NKB = T // 128  # k blocks


# ---------------------------------------------------------------------------
# Compat patches: this container's walrus accepts only ONE sync wait per
# instruction, while Tile freely attaches several. Split excess waits onto
# injected same-engine NOPs / extra drains.
# ---------------------------------------------------------------------------

def _drain_and_barrier_split(self, tick_clock, wait_clock):
    nc = self.nc
    drain_inst = nc.sync.drain()
    wait_clock.add_sem_waits(
        drain_inst.ins, bass_rust.ScopedClock({None: tick_clock.global_clock})
    )
    si = drain_inst.ins.sync_info
    waits = list(si.on_wait or []) if si is not None else []
    if len(waits) > 1:
        si.on_wait = waits[:1]
        for w in waits[1:]:
            extra = nc.sync.drain()
            esi = extra.ins.sync_info
            if esi is None:
                extra.ins.sync_info = bass_rust.SyncInfo(on_wait=[w], on_update=[])
            else:
                esi.on_wait = [w]

    nc.all_engine_barrier()
    assert self.sems is not None
    popped = nc._tile_sem_poison_stack.pop()
    assert popped is self._sem_poison
    nc.clear_and_free_semaphores(list(self.sems.allocated().values()))
    nc.all_engine_barrier()


def _apply_compat_patches():
    tile.TileContext._drain_and_barrier = _drain_and_barrier_split


def _split_excess_waits(nc):
    for fn in nc.m.functions:
        for bb in fn.blocks:
            out = []
            changed = False
            for inst in bb.instructions:
                si = inst.sync_info
                if si is not None and si.on_wait and len(si.on_wait) > 1:
                    waits = list(si.on_wait)
                    for w in waits[:-1]:
                        nop = mybir.InstNoOp(
                            name=f"I-waitsplit-{nc.next_id()}",
                            engine=inst.engine,
                            ins=[],
                            outs=[],
                            sync_info=bass_rust.SyncInfo(on_wait=[w], on_update=[]),
                        )
                        nc.register_instruction(nop)
                        out.append(nop)
                    si.on_wait = waits[-1:]
                    changed = True
                out.append(inst)
            if changed:
                bb.instructions = out


# ---------------------------------------------------------------------------
# Kernel program (identical for all 8 cores; only the bound data differs)
# ---------------------------------------------------------------------------

def _build_nc():
    nc = bass.Bass()
    xT = nc.dram_tensor("xT", [D, T], f32, kind="ExternalInput")
    wqkvT = nc.dram_tensor("wqkvT", [D, OC], f32, kind="ExternalInput")
    woT = nc.dram_tensor("woT", [512, D], f32, kind="ExternalInput")
    trimask = nc.dram_tensor("trimask", [P, P], f32, kind="ExternalInput")
    outT = nc.dram_tensor("outT", [D, T], f32, kind="ExternalOutput")

    xT_r = xT.rearrange("(cs p) t -> p cs t", p=P)          # [128, 8, 2048]
    wqkvT_r = wqkvT.rearrange("(cs p) o -> p cs o", p=P)    # [128, 8, 1536]
    woT_r = woT.rearrange("(cs p) o -> p cs o", p=P)        # [128, 4, 1024]
    outT_r = outT.rearrange("(ob p) t -> p ob t", p=P)      # [128, 8, 2048]

    XC = 256            # t-chunk width for the qkv projection
    NXC = T // XC

    with tile.TileContext(nc) as tc:
        # pool_a: tensors produced in phase 1, consumed in phase 2
        pool_a_cm = tc.tile_pool(name="pool_a", bufs=1)
        pool_a = pool_a_cm.__enter__()
        qT = pool_a.tile([P, HC // 2, T], MDT)      # [128, hp, t] head pair hp
        kT = pool_a.tile([P, HC // 2, T], MDT)
        V = pool_a.tile([P, NKB, HC, DH + 1], MDT)  # [k-part, kb, h, d|1]
        maskr = pool_a.tile([P, P], MDT)

        # ---- phase 0: constants -------------------------------------------
        with tc.tile_pool(name="ph0", bufs=1) as ph0:
            mf = ph0.tile([P, P], f32)
            nc.sync.dma_start(mf[:, :], trimask[:, :])
            nc.vector.tensor_copy(maskr[:, :], mf[:, :])
            onef = ph0.tile([P, 1], f32)
            nc.vector.memset(onef[:, :], 1.0)
            # ones column of V: broadcast along kb/h
            nc.vector.tensor_copy(
                V[:, :, :, DH : DH + 1],
                onef[:, None, None, :].to_broadcast((P, NKB, HC, 1)),
            )

        # ---- phase 1: qkv projections -------------------------------------
        with (
            tc.tile_pool(name="wstage", bufs=1) as wstage,
            tc.tile_pool(name="wpool", bufs=1) as wpool,
            tc.tile_pool(name="xpool", bufs=2) as xpool,
            tc.tile_pool(name="ppsum", bufs=3, space="PSUM") as ppsum,
        ):
            wr = wpool.tile([P, D // P, OC], MDT)
            for i in range(4):
                ws = wstage.tile([P, D // P, OC // 4], f32, tag="ws")
                nc.sync.dma_start(ws[:, :, :], wqkvT_r[:, :, i * (OC // 4):(i + 1) * (OC // 4)])
                nc.vector.tensor_copy(wr[:, :, i * (OC // 4):(i + 1) * (OC // 4)], ws[:, :, :])

            for tcb in range(NXC):
                tlo = tcb * XC
                xs = xpool.tile([P, D // P, XC], f32, tag="xs")
                nc.sync.dma_start(xs[:, :, :], xT_r[:, :, tlo:tlo + XC])
                xr = xpool.tile([P, D // P, XC], MDT, tag="xr")
                nc.vector.tensor_copy(xr[:, :, :], xs[:, :, :])

                # q, k in [o, t] orientation: out rows = features
                for ob in range(8):
                    ps = ppsum.tile([P, XC], f32, tag="pp")
                    for ci in range(D // P):
                        nc.tensor.matmul(
                            ps[:, :],
                            lhsT=wr[:, ci, ob * P:(ob + 1) * P],
                            rhs=xr[:, ci, :],
                            start=(ci == 0),
                            stop=(ci == D // P - 1),
                        )
                    dst = qT if ob < 4 else kT
                    nc.scalar.copy(dst[:, ob % 4, tlo:tlo + XC], ps[:, :])

                # v in [t, o] orientation: out rows = tokens
                for ts in range(XC // P):
                    kb = (tlo + ts * P) // P
                    ps = ppsum.tile([P, 512], f32, tag="pv")
                    for ci in range(D // P):
                        nc.tensor.matmul(
                            ps[:, :],
                            lhsT=xr[:, ci, ts * P:(ts + 1) * P],
                            rhs=wr[:, ci, 1024:1536],
                            start=(ci == 0),
                            stop=(ci == D // P - 1),
                        )
                    nc.vector.tensor_copy(
                        V[:, kb, :, 0:DH],
                        ps.rearrange("p (h d) -> p h d", h=HC),
                    )

        # ---- phase 2: causal attention ------------------------------------
        # persist: tensors needed through phase 3 (opened after phase-1 pools
        # close so yT reuses their SBUF space)
        persist_cm = tc.tile_pool(name="persist", bufs=1)
        persist = persist_cm.__enter__()
        yT = persist.tile([P, HC // 2, T], MDT)
        ones1 = persist.tile([1, DH], f32r)

        with (
            tc.tile_pool(name="ptpool", bufs=3) as ptpool,
            tc.tile_pool(name="bcpool", bufs=2) as bcpool,
            tc.tile_pool(name="rzpool", bufs=2) as rzpool,
            tc.tile_pool(name="spsum", bufs=2, space="PSUM") as spsum,
            tc.tile_pool(name="ypsum", bufs=2, space="PSUM") as ypsum,
            tc.tile_pool(name="bcpsum", bufs=2, space="PSUM") as bcpsum,
        ):
            onef2 = rzpool.tile([1, 1], f32, tag="onef")
            nc.vector.memset(onef2[:, :], 1.0)
            nc.vector.tensor_copy(ones1[:, :], onef2[0:1, 0:1].to_broadcast((1, DH)))
            for hp in range(HC // 2):
                for qc in range(NQC):
                    qlo = qc * 512
                    nkb = 4 * qc + 4
                    psy = [
                        ypsum.tile([DH + 1, 512], f32, tag="psy", name=f"psy_{hp}_{qc}_{hi}")
                        for hi in range(2)
                    ]
                    for kb in range(nkb):
                        j = kb - 4 * qc
                        off = max(0, j * P)
                        ps = spsum.tile([P, 2, 512], f32, tag="ps")
                        for hi in range(2):
                            nc.tensor.matmul(
                                ps[:, hi, off:],
                                lhsT=kT[hi * DH:(hi + 1) * DH, hp, kb * P:(kb + 1) * P],
                                rhs=qT[hi * DH:(hi + 1) * DH, hp, qlo + off:qlo + 512],
                                start=True,
                                stop=True,
                                tile_position=(hi * DH, 0),
                            )
                        pt = ptpool.tile([P, 2, 512], MDT, tag="pt")
                        nc.scalar.activation(pt[:, :, off:], ps[:, :, off:], Exp, scale=0.125)
                        if j >= 0:
                            nc.vector.tensor_mul(
                                pt[:, :, off:off + P],
                                pt[:, :, off:off + P],
                                maskr[:, None, :].to_broadcast((P, 2, P)),
                            )
                        for hi in range(2):
                            h = 2 * hp + hi
                            nc.tensor.matmul(
                                psy[hi][:, off:],
                                lhsT=V[:, kb, h, :],
                                rhs=pt[:, hi, off:],
                                start=(kb == 0),
                                stop=(kb == nkb - 1),
                                skip_group_check=True,
                            )
                    for hi in range(2):
                        rz = rzpool.tile([1, 512], f32r, tag="rz")
                        with nc.allow_low_precision(reason="1/Z rounded for PE broadcast"):
                            nc.vector.reciprocal(rz[:, :], psy[hi][DH:DH + 1, :])
                        bc = bcpsum.tile([DH, 512], f32, tag="bc")
                        nc.tensor.matmul(bc[:, :], lhsT=ones1[:, :], rhs=rz[:, :], start=True, stop=True)
                        bcs = bcpool.tile([DH, 512], f32, tag="bcs")
                        nc.scalar.copy(bcs[:, :], bc[:, :])
                        nc.vector.tensor_mul(
                            yT[hi * DH:(hi + 1) * DH, hp, qlo:qlo + 512],
                            psy[hi][0:DH, :],
                            bcs[:, :],
                        )

        # ---- phase 3: out projection --------------------------------------
        with (
            tc.tile_pool(name="w3stage", bufs=1) as w3stage,
            tc.tile_pool(name="w3pool", bufs=1) as w3pool,
            tc.tile_pool(name="ostage", bufs=2) as ostage,
            tc.tile_pool(name="opsum", bufs=3, space="PSUM") as opsum,
        ):
            w3s = w3stage.tile([P, 4, D], f32)
            nc.sync.dma_start(w3s[:, :, :], woT_r[:, :, :])
            wor = w3pool.tile([P, 4, D], MDT)
            nc.vector.tensor_copy(wor[:, :, :], w3s[:, :, :])

            for tcb in range(NQC):
                tlo = tcb * 512
                ost = ostage.tile([P, 8, 512], f32, tag="ost")
                for ob in range(8):
                    ps = opsum.tile([P, 512], f32, tag="po")
                    for ci in range(4):
                        nc.tensor.matmul(
                            ps[:, :],
                            lhsT=wor[:, ci, ob * P:(ob + 1) * P],
                            rhs=yT[:, ci, tlo:tlo + 512],
                            start=(ci == 0),
                            stop=(ci == 3),
                        )
                    nc.scalar.copy(ost[:, ob, :], ps[:, :])
                nc.sync.dma_start(outT_r[:, :, tlo:tlo + 512], ost[:, :, :])

        persist_cm.__exit__(None, None, None)
        pool_a_cm.__exit__(None, None, None)

    _split_excess_waits(nc)
    return nc


_NC_CACHE = None


def _get_nc():
    global _NC_CACHE
    if _NC_CACHE is None:
        _apply_compat_patches()
        _NC_CACHE = _build_nc()
    return _NC_CACHE


def _shard_inputs(x, w_qkv, w_out):
    trimask = np.triu(np.ones((P, P), dtype=np.float32))
    in_maps = []
    for b in range(B):
        xTb = np.ascontiguousarray(x[b].T)
        for g in range(2):
            wq = w_qkv[g * 512:(g + 1) * 512]
            wk = w_qkv[D + g * 512:D + (g + 1) * 512]
            wv = w_qkv[2 * D + g * 512:2 * D + (g + 1) * 512]
            wqkvT = np.ascontiguousarray(np.concatenate([wq, wk, wv], axis=0).T)
            woT = np.ascontiguousarray(w_out[:, g * 512:(g + 1) * 512].T)
            in_maps.append({"xT": xTb, "wqkvT": wqkvT, "woT": woT, "trimask": trimask})
    return in_maps


def kernel(x, w_qkv, w_out, _trace=False):
    x = np.asarray(x, dtype=np.float32)
    w_qkv = np.asarray(w_qkv, dtype=np.float32)
    w_out = np.asarray(w_out, dtype=np.float32)

    nc = _get_nc()
    in_maps = _shard_inputs(x, w_qkv, w_out)
    res = run_bass_kernel_spmd(nc, in_maps, core_ids=list(range(8)), trace=_trace)

    out = np.empty((B, T, D), dtype=np.float32)
    for b in range(B):
        acc = res.results[2 * b]["outT"] + res.results[2 * b + 1]["outT"]
        out[b] = acc.T
    if _trace:
        return out, res
    return out
